# revision 4
# baseline (speedup 1.0000x reference)
"""DenseQConv1D Trainium2 kernel, v5 — closed-form for the ring-CNOT entangler.

For E = build_entangle(9) (the reference's fixed CNOT-ring permutation) the
measurement operator collapses: with F = E[:128,:256], G = E[:128,256:],
    GX = F G^T + G F^T = 0,          GZ = F F^T - G G^T = diag((-1)^j)
(rows indexed i = c*8+j like the reference's patch vector).  Hence, exactly:
    y[b,l]  = sum_c x[b,c,l]^2
    n2[b,t] = sum_{j<8} y[b,t+j]          (box-8 filter)
    z[b,t]  = sum_{j<8} (-1)^j y[b,t+j]   (alternating-8 filter)
    out[b,c,t] = cos(theta[c,0]) * z[b,t] / n2[b,t]
(validated vs the jax reference at rel err ~9e-7).

Sharding: the 1017 output columns are split into 8 blocks of 128 (last 121);
core k gets ALL batches for its column block as a [128, 135] tile with rows
(b, c_in).  cos(theta) is a degree-5 polynomial in theta^2 on the otherwise
idle gpsimd engine (no ACT trig table needed; the only ACT op is the final
reciprocal).  Output is a [64, 256] tile (two channels per partition row) to
halve output-DMA descriptor count, cos-scaled during PSUM evacuation.
"""

import numpy as np

B = 8
C_IN = 16
C_OUT = 16
L = 1024
K = 8
L_OUT = L - K + 1  # 1017
NCORE = 8
NOUT = 128  # output columns per core (last core uses 121 of them)
NIN = NOUT + K - 1  # 135 input columns per core
P = 128

# degree-5 polynomial in x = t^2: cos(t) for |t| <= 2.9, max err ~1e-6
COS_COEF = [0.9999997815, -0.4999979653, 0.04166362521,
            -0.00138723536, 2.439557794e-05, -2.292118959e-07]

_CACHE = {}


def _build_nc():
    import concourse.bacc as bacc
    import concourse.mybir as mybir
    import concourse.tile as tile

    f32 = mybir.dt.float32
    f32r = mybir.dt.float32r
    AF = mybir.ActivationFunctionType
    ALU = mybir.AluOpType

    nc = bacc.Bacc("TRN2", target_bir_lowering=False, debug=False)

    def act_raw(out, in_, func, bias=0.0, scale=1.0):
        eng = nc.scalar
        bias_arg = (
            eng.lower_ap(bias)
            if not isinstance(bias, float)
            else mybir.ImmediateValue(dtype=mybir.dt.float32, value=bias)
        )
        ins = [
            eng.lower_ap(in_),
            bias_arg,
            mybir.ImmediateValue(dtype=mybir.dt.float32, value=scale),
            mybir.ImmediateValue(dtype=mybir.dt.float32, value=0.0),
        ]
        return eng.add_instruction(
            mybir.InstActivation(
                name=nc.get_next_instruction_name(), func=func,
                ins=ins, outs=[eng.lower_ap(out)],
            )
        )

    xs_ext = nc.declare_dram_parameter("xs", [64, 2 * NIN], f32, isOutput=False)
    th_ext = nc.declare_dram_parameter("thc", [C_OUT, 2], f32, isOutput=False)
    out_ext = nc.declare_dram_parameter("out", [64, 2 * NOUT], f32, isOutput=True)

    with tile.TileContext(nc) as tc, \
            tc.tile_pool(name="const", bufs=1) as const, \
            tc.tile_pool(name="sb", bufs=1) as sb, \
            tc.tile_pool(name="ps", bufs=1, space="PSUM") as psp:
        xs = sb.tile([64, 2 * NIN], f32)
        thc = sb.tile([C_OUT, 2], f32)

        # xs halves split across the two HWDGE rings; theta follows on
        # the scalar ring (the cos chain has ~1.5us of slack)
        nc.sync.dma_start(xs[0:32, :], xs_ext[0:32, :])
        nc.scalar.dma_start(xs[32:64, :], xs_ext[32:64, :])
        nc.scalar.dma_start(thc[:], th_ext[:])

        # ---- constants (gpsimd), ordered by first use ----
        fill0 = nc.gpsimd.to_reg(0.0)
        fill1 = nc.gpsimd.to_reg(1.0)
        # BO[p, b'] = 1 iff p//8 == b'   (64 x 8 block-column mask; input
        # rows are (b, c2) with two channels folded into the free axis)
        BO = const.tile([64, 8], f32)
        nc.gpsimd.memset(BO[:], 1.0)
        nc.gpsimd.affine_select(
            out=BO[:], in_=BO[:], compare_op=ALU.is_ge, fill=fill0,
            base=0, pattern=[[-8, 8]], channel_multiplier=1,
        )
        nc.gpsimd.affine_select(
            out=BO[:], in_=BO[:], compare_op=ALU.is_ge, fill=fill0,
            base=7, pattern=[[8, 8]], channel_multiplier=-1,
        )
        # MC2[c, (b,k)]: cols 0-63 select even channels (c == 2k), cols
        # 64-127 odd (c == 2k+1); the b index is a stride-0 pattern dim.
        MC2 = const.tile([C_IN, P], f32)
        nc.gpsimd.memset(MC2[:], 0.0)
        nc.gpsimd.affine_select(
            out=MC2[:, 0:64], in_=MC2[:, 0:64],
            compare_op=ALU.not_equal, fill=fill1,
            base=0, pattern=[[0, 8], [-2, 8]], channel_multiplier=1,
        )
        nc.gpsimd.affine_select(
            out=MC2[:, 64:P], in_=MC2[:, 64:P],
            compare_op=ALU.not_equal, fill=fill1,
            base=-1, pattern=[[0, 8], [-2, 8]], channel_multiplier=1,
        )
        # W64[b', (b,k)] = 1 iff b' == b   (8 x 64 block-row mask)
        W64 = const.tile([8, 64], f32)
        nc.gpsimd.memset(W64[:], 1.0)
        nc.gpsimd.affine_select(
            out=W64[:], in_=W64[:], compare_op=ALU.is_ge, fill=fill0,
            base=0, pattern=[[1, 64]], channel_multiplier=-8,
        )
        nc.gpsimd.affine_select(
            out=W64[:], in_=W64[:], compare_op=ALU.is_ge, fill=fill0,
            base=7, pattern=[[-1, 64]], channel_multiplier=8,
        )

        # W = [W0 | W1] with 40 output rows each: box filter -> PSUM rows
        # 0-7, alternating filter -> rows 32-39 (reads must start at a
        # partition quadrant).  Dead cols filled with BO (rows never read).
        W = sb.tile([64, 80], f32r)
        nc.vector.tensor_copy(W[:, 0:8], BO[:])
        nc.vector.tensor_copy(W[:, 32:40], BO[:])
        nc.vector.tensor_copy(W[:, 40:48], BO[:])
        nc.vector.tensor_scalar_mul(W[:, 72:80], BO[:], -1.0)
        for c0 in (8, 16, 24, 48, 56, 64):
            nc.vector.tensor_copy(W[:, c0 : c0 + 8], BO[:])
        # DVE-rounded f32r copies of the gpsimd masks used as matmul weights
        MC2r = sb.tile([C_IN, P], f32r)
        nc.vector.tensor_copy(MC2r[:], MC2[:])
        W64r = sb.tile([8, 64], f32r)
        nc.vector.tensor_copy(W64r[:], W64[:])

        # dummy reciprocal on a constant: the first ACT instruction, so the
        # reciprocal table prefetches immediately; Square/Copy below also
        # live in that table, so no further table loads ever happen
        dz = sb.tile([1, 2], f32)
        nc.vector.memset(dz[:], 1.0)
        dzo = sb.tile([1, 2], f32)
        act_raw(dzo[:], dz[:], AF.Reciprocal)

        # ---- cos(theta) on ACT via iterated affine-squares (max err 3e-6
        # for |t| <= 2.9).  Square/Copy live in the same ACT table as
        # Reciprocal, so the whole kernel needs a single table family and
        # the table queue never thrashes. ----
        CQ = [0.016203629623126187, -1.4420320349422868,
              -0.9882186745906292, 0.5363028899458568,
              -1.2270615909104523, 1.4530563055120675,
              1.0548677666154753, -0.9999652180452864]
        q0 = sb.tile([C_OUT, 2], f32)
        act_raw(q0[:], thc[:], AF.Square)
        g1 = sb.tile([C_OUT, 2], f32)
        act_raw(g1[:], q0[:], AF.Square, bias=CQ[1], scale=CQ[0])
        g2 = sb.tile([C_OUT, 2], f32)
        act_raw(g2[:], g1[:], AF.Square, bias=CQ[3], scale=CQ[2])
        g3 = sb.tile([C_OUT, 2], f32)
        act_raw(g3[:], g2[:], AF.Square, bias=CQ[5], scale=CQ[4])
        cs = sb.tile([C_OUT, 2], f32r)
        act_raw(cs[:], g3[:], AF.Copy, bias=CQ[7], scale=CQ[6])
        # zero bias derived from q0 pins the reciprocal after the start of
        # the cos chain in the in-order ACT pipe (cheap DVE slot while the
        # PE runs the filter matmuls)
        bias0 = sb.tile([8, 1], f32)
        nc.vector.tensor_scalar_mul(bias0[:], q0[0:8, 0:1], 0.0)

        # ---- main chain ----
        xsq = sb.tile([64, 2 * NIN], f32r)
        nc.vector.tensor_mul(xsq[:], xs[:], xs[:])

        # BA2 rows: 0-7 = y[t]+y[t+1] (box), 32-39 = y[t]-y[t+1] (alt);
        # each weight set accumulates the even- and odd-channel col halves
        ba_ps = psp.tile([40, NIN - 1], f32, tag="ba")
        nc.tensor.matmul(
            ba_ps[:], W[:, 0:40], xsq[:, 0 : NIN - 1], start=True, stop=False
        )
        nc.tensor.matmul(
            ba_ps[:], W[:, 0:40], xsq[:, NIN : 2 * NIN - 1],
            start=False, stop=False,
        )
        nc.tensor.matmul(
            ba_ps[:], W[:, 40:80], xsq[:, 1:NIN], start=False, stop=False
        )
        nc.tensor.matmul(
            ba_ps[:], W[:, 40:80], xsq[:, NIN + 1 : 2 * NIN],
            start=False, stop=True,
        )

        # csEO[(b,k)] = cos(theta[2k]) rows 0-63, cos(theta[2k+1]) rows 64-127
        cscol_ps = psp.tile([P, 2], f32, tag="cscol")
        nc.tensor.matmul(cscol_ps[:], MC2r[:], cs[:], start=True, stop=True)

        # combined 40-row evac + 4-tap, then 8-tap box first (feeds the ACT
        # reciprocal) with the alternating 8-tap overlapping the reciprocal
        BA2 = sb.tile([40, NIN - 1], f32)
        nc.vector.tensor_copy(BA2[:], ba_ps[:])
        BA4 = sb.tile([40, NIN - 3], f32)
        nc.vector.tensor_add(BA4[:], BA2[:, 0 : NIN - 3], BA2[:, 2 : NIN - 1])
        B8 = sb.tile([8, NOUT], f32)
        nc.vector.tensor_add(B8[:], BA4[0:8, 0:NOUT], BA4[0:8, 4 : NIN - 3])
        inv = sb.tile([8, NOUT], f32)
        act_raw(inv[:], B8[:], AF.Reciprocal, bias=bias0[:])
        A8 = sb.tile([8, NOUT], f32)
        nc.vector.tensor_add(
            A8[:], BA4[32:40, 0:NOUT], BA4[32:40, 4 : NIN - 3]
        )

        zn = sb.tile([8, NOUT], f32r)
        nc.vector.tensor_mul(zn[:], A8[:], inv[:])

        csEO = sb.tile([P, 2], f32)
        nc.scalar.activation(csEO[:], cscol_ps[:], AF.Copy)

        # out rows (b,k) hold channels 2k (cols 0:128) and 2k+1 (128:256);
        # cos scaling is applied per-partition during PSUM evacuation.
        outs = sb.tile([64, 2 * NOUT], f32)
        opE = psp.tile([64, NOUT], f32, tag="opE")
        nc.tensor.matmul(opE[:], W64r[:], zn[:], start=True, stop=True)
        nc.vector.tensor_scalar(
            outs[:, 0:NOUT], opE[:], csEO[0:64, 0:1], None, op0=ALU.mult
        )
        nc.scalar.activation(
            outs[:, NOUT : 2 * NOUT], opE[:], AF.Copy,
            scale=csEO[64:P, 0:1],
        )
        # split the store by ROWS: each descriptor covers a full contiguous
        # 1KB DRAM row, halving per-queue descriptor-generation time vs a
        # column split (which would give 512B descriptors)
        nc.sync.dma_start(out_ext[0:32, :], outs[0:32, :])
        nc.scalar.dma_start(out_ext[32:64, :], outs[32:64, :])

    nc.compile()
    return nc


def _make_in_maps(x, theta):
    """Host-side sharding: core k gets all batches for output cols
    [128k, 128k+128) as a [128, 135] tile (rows b*16+c), one-padded past
    the end of x."""
    thc = np.ascontiguousarray(
        np.repeat(theta[:, 0:1], 2, axis=1), dtype=np.float32
    )
    xpad = np.ones((B, C_IN, NCORE * NOUT + K - 1), dtype=np.float32)
    xpad[:, :, :L] = x
    in_maps = []
    for k in range(NCORE):
        lo = k * NOUT
        blk = xpad[:, :, lo : lo + NIN].reshape(B, 8, 2, NIN)
        xs = np.ascontiguousarray(
            np.concatenate([blk[:, :, 0, :], blk[:, :, 1, :]], axis=-1)
            .reshape(64, 2 * NIN),
            dtype=np.float32,
        )
        in_maps.append({"xs": xs, "thc": thc})
    return in_maps


def _assemble(results):
    """res["out"] is [64, 256]: row 8b+k, col 128h+t -> out[b, 2k+h, t]."""
    out = np.empty((B, C_OUT, L_OUT), dtype=np.float32)
    for k in range(NCORE):
        lo = k * NOUT
        nk = min(NOUT, L_OUT - lo)
        blk = (
            results[k]["out"]
            .reshape(B, 8, 2, NOUT)
            .reshape(B, C_OUT, NOUT)
        )
        out[:, :, lo : lo + nk] = blk[:, :, :nk]
    return out


def kernel(**inputs):
    from concourse.bass_utils import run_bass_kernel_spmd

    x = np.ascontiguousarray(np.asarray(inputs["x"], dtype=np.float32))
    theta = np.ascontiguousarray(np.asarray(inputs["theta"], dtype=np.float32))

    if "nc" not in _CACHE:
        _CACHE["nc"] = _build_nc()
    nc = _CACHE["nc"]

    in_maps = _make_in_maps(x, theta)
    res = run_bass_kernel_spmd(nc, in_maps, core_ids=list(range(NCORE)))
    return _assemble(res.results)
